# revision 1
# baseline (speedup 1.0000x reference)
"""BiLSTM Trainium2 kernel — 8 NeuronCores, SPMD.

Sharding: data-parallel over batch x direction. Cores 0-3 run the forward
LSTM on batch shards of 16; cores 4-7 run the SAME SPMD graph on
time-reversed inputs with the backward weights (direction is pure input
data). The tag projection is split across directions: fwd cores apply
W_tag[:256], bwd cores W_tag[256:]; the host sums the two partials.

Device layout (per core):
  - everything lives transposed ("gates^T" layout): gate-chunk dim on the
    128 SBUF partitions, batch on the free dim. The recurrence
    gates^T = Wh^T @ h^T keeps Wh tiles stationary on the PE array and h^T
    moving, and produces h^T directly -> no per-step transposes.
  - gate chunks are permuted to slots [g0,g1, i0,i1, f0,f1, o0,o1]; all 8
    slots of one step land in ONE PSUM bank, the precomputed input
    projection is accumulated into it by an identity matmul, and a single
    sigmoid covers every gate (g weights pre-scaled x2 on the host:
    tanh(x) = 2*sigmoid(2x) - 1, exact in bf16).
  - input projection xp = (emb@Wx + b)^T is precomputed for all 256 steps
    as one big matmul (bf16), then the 256-step recurrence runs with
    fp32 cell state and bf16 hidden state.
  - this stack's walrus rejects instructions carrying >1 semaphore wait;
    _legalize_bir_waits post-processes Tile's BIR to hoist extra waits
    onto standalone EventSemaphore instructions.
"""

import json
import os
import sys
import types
import numpy as np
import ml_dtypes

for _p in ("/root/.axon_site/_ro/trn_rl_repo", "/opt/trn_rl_repo"):
    if _p not in sys.path and os.path.isdir(_p):
        sys.path.append(_p)


def _ensure_ntff_hook():
    """This image's antenv lacks axon_hooks; synthesize it so
    run_bass_kernel_spmd(trace=True) can reach the NTFF profiler."""
    try:
        import antenv.axon_hooks  # noqa: F401
        return
    except ImportError:
        pass
    try:
        import antenv
        from trn_agent_boot.trn_boot import _ntff_profile_via_ctypes
        mod = types.ModuleType("antenv.axon_hooks")
        _hook = [None]

        def set_axon_ntff_profile_hook(h):
            _hook[0] = h

        def get_axon_ntff_profile_hook():
            if _hook[0] is None:
                try:
                    _hook[0] = _ntff_profile_via_ctypes("/opt/axon/libaxon_pjrt.so")
                except Exception:
                    return None
            return _hook[0]

        mod.set_axon_ntff_profile_hook = set_axon_ntff_profile_hook
        mod.get_axon_ntff_profile_hook = get_axon_ntff_profile_hook
        sys.modules["antenv.axon_hooks"] = mod
        antenv.axon_hooks = mod
    except Exception:
        pass


_ensure_ntff_hook()

import concourse.bass as bass
import concourse.tile as tile
from concourse import mybir
from concourse.bass_utils import run_bass_kernel_spmd


def _enable_ldw_opt():
    """Flip walrus --enable-ldw-opt=false -> true. DISABLED by default:
    this walrus build's visitInstLdweights faults with the opt on."""
    if not os.environ.get("BILSTM_LDW_OPT"):
        return
    from concourse import bass_utils as _bu
    if getattr(_bu, "_ldw_patched", False):
        return
    orig = _bu.run_command

    def patched(cmd, **kw):
        cmd = [c.replace("--enable-ldw-opt=false", "--enable-ldw-opt=true")
               if isinstance(c, str) else c for c in cmd]
        return orig(cmd, **kw)

    _bu.run_command = patched
    _bu._ldw_patched = True


_enable_ldw_opt()

BF16 = ml_dtypes.bfloat16
F32 = mybir.dt.float32
BF = mybir.dt.bfloat16
AF = mybir.ActivationFunctionType

E, H2, TAGS = 256, 256, 20
S = 256          # sequence length
B = 64           # global batch
BC = 16          # batch per core
KC = 2           # contraction chunks (E = H2 = 256 -> 2 x 128)
NB = 8           # free-dim blocks for the big matmuls
W = S * BC // NB  # 512
# slot -> original gate chunk (orig gate order i,f,g,o; 2 chunks each)
PERM = [4, 5, 0, 1, 2, 3, 6, 7]  # [g0,g1, i0,i1, f0,f1, o0,o1]

_CACHE = {}
LAST_RESULT = None  # test harness introspection


def _legalize_bir_waits(raw):
    """This stack's walrus rejects any instruction carrying >=2 semaphore
    waits ("Too many sync wait commands"). Split such waits onto standalone
    single-wait EventSemaphore instructions inserted just before, on the
    same engine — semantically identical (engine streams are in-order)."""
    d = json.loads(raw)
    n = 0
    for fn in d.get("functions", []):
        for bb in fn.get("blocks", []):
            out = []
            for inst in bb.get("instructions", []):
                si = inst.get("sync_info") or {}
                waits = si.get("on_wait") or []
                if len(waits) >= 2:
                    for w_ in waits[:-1]:
                        n += 1
                        out.append({
                            "debug": inst.get("debug", 0),
                            "engine": inst["engine"],
                            "ins": [], "outs": [],
                            "name": f"legw-{n}",
                            "opcode": "EventSemaphore",
                            "sync_info": {"on_update": [], "on_wait": [w_]},
                        })
                    si = dict(si)
                    si["on_wait"] = [waits[-1]]
                    inst = dict(inst)
                    inst["sync_info"] = si
                out.append(inst)
            bb["instructions"] = out
    return json.dumps(d).encode()


def _build(S=S, with_bias=True):
    W = S * BC // NB
    nc = bass.Bass()
    xsT_e = nc.declare_dram_parameter("xsT", [E, S * BC], BF, isOutput=False)
    wx_e = nc.declare_dram_parameter("wx", [128, 8, KC, 128], BF, isOutput=False)
    wh_e = nc.declare_dram_parameter("wh", [128, 8, KC, 128], BF, isOutput=False)
    b_e = nc.declare_dram_parameter("bvec", [128, 8], F32, isOutput=False)
    wt_e = nc.declare_dram_parameter("wtag", [128, KC, TAGS], BF, isOutput=False)
    bt_e = nc.declare_dram_parameter("btag", [TAGS, 1], F32, isOutput=False)
    id_e = nc.declare_dram_parameter("ident", [128, 128], BF, isOutput=False)
    out_e = nc.declare_dram_parameter("outT", [TAGS, S * BC], F32, isOutput=True)

    with tile.TileContext(nc) as tc:
        with (
            tc.tile_pool(name="big", bufs=1) as big,
            tc.tile_pool(name="small", bufs=2) as small,
            tc.tile_pool(name="mm_psum", bufs=4, space="PSUM") as mmp,
            tc.tile_pool(name="rec_psum", bufs=2, space="PSUM") as rp,
        ):
            xs = big.tile([128, KC, S * BC], BF)        # xs^T (E on partitions)
            xp = big.tile([128, 8, S * BC], BF)         # xproj^T, slot-major
            # h^T history: [p, step, kc*BC+b]; step 0 = h_{-1} = 0
            hst = big.tile([128, S + 1, KC * BC], BF)
            wx = big.tile([128, 8, KC, 128], BF)
            wh = big.tile([128, 8, KC, 128], BF)
            bv = big.tile([128, 8], F32)
            wt = big.tile([128, KC, TAGS], BF)
            bt = big.tile([TAGS, 1], F32)
            ident = big.tile([128, 128], BF)
            cst = big.tile([128, KC * BC], F32)         # cell state c^T (fp32)
            outb = big.tile([TAGS, S * BC], F32)

            for kc in range(KC):
                for h in range(2):
                    HW_ = S * BC // 2
                    nc.gpsimd.dma_start(
                        xs[:, kc, h * HW_:(h + 1) * HW_],
                        xsT_e[kc * 128:(kc + 1) * 128, h * HW_:(h + 1) * HW_],
                    )
            nc.gpsimd.dma_start(wx[:], wx_e[:])
            nc.gpsimd.dma_start(wh[:], wh_e[:])
            nc.gpsimd.dma_start(bv[:], b_e[:])
            nc.gpsimd.dma_start(wt[:], wt_e[:])
            nc.gpsimd.dma_start(bt[:], bt_e[:])
            nc.gpsimd.dma_start(ident[:], id_e[:])

            nc.vector.memset(hst[:, 0, :], 0.0)
            nc.vector.memset(cst[:], 0.0)
            # warm the ACT table (sigmoid_and_others includes tanh)
            warm = small.tile([128, 8], F32, tag="warm")
            nc.scalar.activation(warm[:], bv[:], AF.Sigmoid)

            # ---- input projection: xp[slot, t*BC+b] = (xs @ Wx + b)^T ----
            for slot in range(8):
                for nb in range(NB):
                    px = mmp.tile([128, W], F32, tag="px")
                    for kc in range(KC):
                        nc.tensor.matmul(
                            px[:],
                            lhsT=wx[:, slot, kc, :],
                            rhs=xs[:, kc, nb * W:(nb + 1) * W],
                            start=(kc == 0),
                            stop=(kc == KC - 1),
                        )
                    dst_ap = xp[:, slot, nb * W:(nb + 1) * W]
                    if with_bias:
                        nc.vector.tensor_add(
                            dst_ap, px[:],
                            bv[:, slot:slot + 1].broadcast_to([128, W]),
                        )
                    elif (slot * NB + nb) % 2 == 0:
                        nc.vector.tensor_copy(dst_ap, px[:])
                    else:
                        nc.scalar.copy(dst_ap, px[:])

            # ---- recurrence ----
            # one PSUM bank per step: cols [g(32) | i(32) f(32) o(32)] x BC
            for t in range(S):
                pall = rp.tile([128, 8 * BC], F32, tag="pall")
                # xp accumulated into PSUM via identity matmul (replaces DVE adds)
                nc.tensor.matmul(
                    pall[:],
                    lhsT=ident[:],
                    rhs=xp[:, :, t * BC:(t + 1) * BC],
                    start=True, stop=False, skip_group_check=True,
                )
                # kc-major: the 8 kc=0 matmuls only need the first half of h,
                # which the epilogue writes ~200ns before the second half
                for kc in range(KC):
                    for slot in range(8):
                        nc.tensor.matmul(
                            pall[:, slot * BC:(slot + 1) * BC],
                            lhsT=wh[:, slot, kc, :],
                            rhs=hst[:, t, kc * BC:(kc + 1) * BC],
                            start=False, stop=(slot == 7 and kc == KC - 1),
                            skip_group_check=True,
                        )
                # g-gate weights are pre-scaled x2 on host: tanh(x) = 2*sig(2x)-1,
                # so ONE sigmoid covers all 8 gate chunks.
                sall = small.tile([128, 8 * BC], F32, tag="sall")
                g2 = small.tile([128, 2 * BC], F32, tag="g2")
                ig = small.tile([128, 2 * BC], F32, tag="ig")
                fc = small.tile([128, 2 * BC], F32, tag="fc")
                tch = small.tile([128, 2 * BC], F32, tag="tch")

                nc.scalar.activation(sall[:], pall[:], AF.Sigmoid)
                nc.vector.tensor_scalar(
                    g2[:], sall[:, 0:2 * BC], 2.0, -1.0,
                    mybir.AluOpType.mult, mybir.AluOpType.add,
                )
                nc.vector.tensor_mul(fc[:], sall[:, 4 * BC:6 * BC], cst[:])
                nc.vector.tensor_mul(ig[:], sall[:, 2 * BC:4 * BC], g2[:])
                nc.vector.tensor_add(cst[:], ig[:], fc[:])
                nc.scalar.activation(tch[:], cst[:], AF.Tanh)
                nc.vector.tensor_mul(hst[:, t + 1, 0:BC], sall[:, 6 * BC:7 * BC], tch[:, 0:BC])
                nc.vector.tensor_mul(hst[:, t + 1, BC:2 * BC], sall[:, 7 * BC:8 * BC], tch[:, BC:2 * BC])

            # ---- tag projection (half of W_tag; host sums fwd+bwd) ----
            # small chunks (N=128) so interleaved scheduling can't block the
            # recurrence chain for long
            WO = 128
            for nb in range(S * BC // WO):
                pt = rp.tile([128, WO], F32, tag="pt")
                for kc in range(KC):
                    nc.tensor.matmul(
                        pt[0:TAGS, :],
                        lhsT=wt[:, kc, :],
                        rhs=hst[:, 1 + nb * (WO // BC):1 + (nb + 1) * (WO // BC), kc * BC:(kc + 1) * BC],
                        start=(kc == 0),
                        stop=(kc == KC - 1),
                    )
                if with_bias:
                    nc.vector.tensor_add(
                        outb[:, nb * WO:(nb + 1) * WO], pt[0:TAGS, :],
                        bt[:, 0:1].broadcast_to([TAGS, WO]),
                    )
                else:
                    nc.vector.tensor_copy(outb[:, nb * WO:(nb + 1) * WO], pt[0:TAGS, :])
            for h in range(2):
                HW_ = S * BC // 2
                nc.gpsimd.dma_start(out_e[:, h * HW_:(h + 1) * HW_], outb[:, h * HW_:(h + 1) * HW_])
    return nc


def _prep_w(Wmat):
    """[256, 1024] -> [128 part, slot 8, kc 2, m 128] bf16, slot-permuted.
    g-gate slots (0,1) are scaled x2: the kernel computes tanh(x) as
    2*sigmoid(2x)-1 (exact; x2 only bumps the bf16 exponent)."""
    t = Wmat.reshape(KC, 128, 8, 128)[:, :, PERM, :].astype(np.float32).copy()
    t[:, :, 0:2, :] *= 2.0
    return np.ascontiguousarray(t.transpose(1, 2, 0, 3)).astype(BF16)


def _prep_b(b):
    """[1024] -> [128, 8] f32, slot-permuted per-partition bias (g x2)."""
    b8 = b.reshape(8, 128)[PERM, :].astype(np.float32).copy()
    b8[0:2, :] *= 2.0
    return np.ascontiguousarray(b8.T)


def kernel(x, emb, Wx_f, Wh_f, b_f, Wx_b, Wh_b, b_b, W_tag, b_tag):
    x = np.asarray(x)
    emb = np.asarray(emb, np.float32)
    Wx_f, Wh_f, b_f = (np.asarray(a, np.float32) for a in (Wx_f, Wh_f, b_f))
    Wx_b, Wh_b, b_b = (np.asarray(a, np.float32) for a in (Wx_b, Wh_b, b_b))
    W_tag = np.asarray(W_tag, np.float32)
    b_tag = np.asarray(b_tag, np.float32)

    with_bias = bool(b_f.any() or b_b.any() or b_tag.any())
    key = ("nc", with_bias)
    if key not in _CACHE:
        nc = _build(with_bias=with_bias)
        legalized = _legalize_bir_waits(nc.to_json_bytes())
        nc.to_json_bytes = lambda: legalized  # shadow: feed legalized BIR to compile
        _CACHE[key] = nc
    nc = _CACHE[key]

    embeds = emb[x]  # [B, S, E] f32
    in_maps = []
    for core in range(8):
        fwd = core < 4
        c = core % 4
        eb = embeds[c * BC:(c + 1) * BC]  # [BC, S, E]
        if not fwd:
            eb = eb[:, ::-1, :]
        xsT = np.ascontiguousarray(eb.transpose(2, 1, 0).reshape(E, S * BC)).astype(BF16)
        Wx, Wh, bb = (Wx_f, Wh_f, b_f) if fwd else (Wx_b, Wh_b, b_b)
        wth = W_tag[:H2] if fwd else W_tag[H2:]
        wt_d = np.ascontiguousarray(wth.reshape(KC, 128, TAGS).transpose(1, 0, 2)).astype(BF16)
        bt_d = (b_tag if fwd else np.zeros_like(b_tag)).reshape(TAGS, 1).astype(np.float32)
        in_maps.append({
            "xsT": xsT,
            "wx": _prep_w(Wx),
            "wh": _prep_w(Wh),
            "bvec": _prep_b(bb),
            "wtag": wt_d,
            "btag": bt_d,
            "ident": np.eye(128, dtype=BF16),
        })

    trace = bool(os.environ.get("BILSTM_TRACE"))
    global LAST_RESULT
    kw = {}
    if trace:
        kw["tmpdir"] = os.environ.get("BILSTM_TRACE_DIR", "/tmp/bilstm_trace")
        os.makedirs(kw["tmpdir"], exist_ok=True)
    res = run_bass_kernel_spmd(nc, in_maps, core_ids=list(range(8)), trace=trace, **kw)
    LAST_RESULT = res

    outs = [np.asarray(res.results[i]["outT"], np.float32).reshape(TAGS, S, BC) for i in range(8)]
    out = np.empty((B, S, TAGS), np.float32)
    for c in range(4):
        tot = outs[c] + outs[c + 4][:, ::-1, :]
        out[c * BC:(c + 1) * BC] = tot.transpose(2, 1, 0)
    return out

